# revision 3
# baseline (speedup 1.0000x reference)
"""Bi-directional minGRU Trainium2 kernel.

Full-input contract: kernel(**inputs) takes the unsharded numpy inputs from
reference.setup_inputs() and returns the full (B, L, 1) float32 output.

Sharding: data-parallel over batch B=32 across 8 NeuronCores (4 sequences per
core), parameters replicated. Per core, each sequence is processed in a
feature-on-partition / time-on-free layout:

  t_enc  : relu(t*w1+b1) via ScalarE (per-partition scale/bias), @w2 on PE
  xc     : [t_enc(64) ; x(2) ; ones(1)] -> 67 partitions (ones row folds the
           proj bias into the matmul)
  inp    : xc @ [proj_w;proj_b] on PE (2 p-halves)
  z,hb   : inp @ wz / wh on PE (2 k-tiles), sigmoid/tanh+bias on ScalarE
  a=1-z  : DVE tensor_scalar;  b=z*hb : DVE tensor_tensor
  scan   : DVE tensor_tensor_scan (state = a*state + b), shifted one step
           (reference stores pre-update state); backward direction runs the
           same scan through negative-stride APs
  head   : [h_fwd;h_bwd;t_enc] @ gh_w1 (5 k-tiles) + relu+bias on ScalarE,
           @ gh_w2 on PE; gh_b2 is added on host after the gather.
"""

import time

import numpy as np
import ml_dtypes

import concourse.bass as bass
import concourse.mybir as mybir
import concourse.tile as tile
from concourse.vector_clock import ScopedClock, VectorClock
from concourse.bass_utils import run_bass_kernel_spmd

# ---------------------------------------------------------------------------
# Workaround for a walrus codegen limit in this toolchain: the TileContext
# tail drain carries one sync-wait per live proc sem, but this walrus build
# rejects >2 sync waits on a Drain (CTRL_NO_STRUCT template). Re-emit the tail
# with the waits split across single-wait NOPs on the sync engine (same-engine
# program order preserves the semantics), followed by a wait-free drain.
# ---------------------------------------------------------------------------


def _patched_drain_and_barrier(self, tick_clock, wait_clock):
    nc = self.nc
    vals = list(tick_clock.global_clock)
    n = len(vals)
    for i, v in enumerate(vals):
        if v > 0:
            partial = [0] * n
            partial[i] = v
            nop = nc.sync.nop()
            wait_clock.add_sem_waits(nop.ins, ScopedClock({None: VectorClock(partial)}))
    nc.sync.drain()
    nc.all_engine_barrier()
    assert self.sems is not None
    popped = nc._tile_sem_poison_stack.pop()
    assert popped is self._sem_poison
    nc.clear_and_free_semaphores(list(self.sems.allocated().values()))
    nc.all_engine_barrier()


tile.TileContext._drain_and_barrier = _patched_drain_and_barrier


def _spill_excess_waits(nc, maxw=1):
    """Split instructions carrying more than `maxw` sem waits: the excess
    waits move onto NoOps inserted just before, on the same engine (same-
    engine program order keeps the semantics identical)."""
    for bb in nc.m.functions[0].blocks:
        new = []
        for inst in bb.instructions:
            si = inst.sync_info
            if si is not None and si.on_wait is not None and len(si.on_wait) > maxw:
                waits = list(si.on_wait)
                excess, keep = waits[:-maxw], waits[-maxw:]
                for j, w in enumerate(excess):
                    nop = mybir.InstNoOp(
                        name=f"{inst.name}_ws{j}",
                        engine=inst.engine,
                        ins=[],
                        outs=[],
                        sync_info=mybir.SyncInfo(on_wait=[w], on_update=[]),
                    )
                    nc.register_instruction(nop)
                    new.append(nop)
                si.on_wait = keep
            new.append(inst)
        if len(new) != len(bb.instructions):
            _replace_block_instructions(bb, new)


def _replace_block_instructions(bb, new):
    try:
        bb.instructions = new
    except Exception:
        while len(bb.instructions):
            bb.instructions.pop()
        for inst in new:
            bb.add_instruction(inst)

# ---------------------------------------------------------------------------

B, L, H, TE = 32, 2048, 256, 64
NCORES = 8
BS = B // NCORES           # sequences per core
HH = 128                   # gauss head hidden
IN_AUG = TE + 2 + 1        # xc rows: t_enc(64) + x(2) + ones(1)
F32 = mybir.dt.float32

DT = mybir.dt.bfloat16     # matmul/activation storage dtype
NP_DT = ml_dtypes.bfloat16

FCH = 512                  # matmul moving-operand chunk (one PSUM bank fp32)
NCH = L // FCH


def _rev(t, cols, ncols):
    """Reversed-free-dim view of tile AP t over columns [cols, cols+ncols)."""
    return bass.AP(
        tensor=t.tensor,
        offset=t.offset + cols + ncols - 1,
        ap=[list(t.ap[0]), [-1, ncols]],
    )


def _build_nc(bs=BS, repeats=1, psum_cols=1024, psum_bufs=4, mp_bufs=2, hp_bufs=3,
              copy_engine="any", ab_engine="vector", bwd_scan_engine="vector",
              te1_mode="dve", r_engine="act", orow_engine="any"):
    nc = bass.Bass("TRN2", target_bir_lowering=False, debug=False, num_devices=NCORES)

    d_xT = nc.dram_tensor("xT", [bs, 2, L], DT, kind="ExternalInput")
    d_t = nc.dram_tensor("t", [bs, L], DT, kind="ExternalInput")
    # gate weights with the input projection and time-encoder composed in
    # (host-side): operate directly on R = [te1_hidden(64); x(2); ones(1)]
    d_wz = {d: nc.dram_tensor(f"wz{d}", [IN_AUG, H], DT, kind="ExternalInput")
            for d in "fb"}
    d_wh = {d: nc.dram_tensor(f"wh{d}", [IN_AUG, H], DT, kind="ExternalInput")
            for d in "fb"}
    d_bz = {d: nc.dram_tensor(f"bz{d}", [H, 1], F32, kind="ExternalInput")
            for d in "fb"}
    d_bh = {d: nc.dram_tensor(f"bh{d}", [H, 1], F32, kind="ExternalInput")
            for d in "fb"}
    d_g1 = nc.dram_tensor("g1", [2 * H, HH], DT, kind="ExternalInput")
    d_g1te = nc.dram_tensor("g1te", [IN_AUG, HH], DT, kind="ExternalInput")
    d_g2 = nc.dram_tensor("g2", [HH, 1], DT, kind="ExternalInput")
    d_tw1 = nc.dram_tensor("tw1", [TE, 1], F32, kind="ExternalInput")
    d_tb1 = nc.dram_tensor("tb1", [TE, 1], F32, kind="ExternalInput")
    d_gb1 = nc.dram_tensor("gb1", [HH, 1], F32, kind="ExternalInput")
    d_out = nc.dram_tensor("out", [bs, L], F32, kind="ExternalOutput")

    with tile.TileContext(nc) as tc:
        with (
            tc.tile_pool(name="wpool", bufs=1) as wp,
            tc.tile_pool(name="mpool", bufs=mp_bufs) as mp,
            tc.tile_pool(name="hpool", bufs=hp_bufs) as hp,
            tc.tile_pool(name="psum", bufs=psum_bufs, space="PSUM") as pp,
        ):
            # ---- replicated weights, loaded once ----
            # DMAs round-robin across engine queues, ordered by first use so
            # the first batch's gates aren't gated on the full weight load.
            _eng = [nc.sync, nc.gpsimd, nc.scalar]
            _ei = [0]

            def wload(shape, dtype, tag, src_ap):
                t = wp.tile(shape, dtype, tag=tag, name=tag)
                _eng[_ei[0] % len(_eng)].dma_start(out=t, in_=src_ap)
                _ei[0] += 1
                return t

            s_tw1 = wload([TE, 1], F32, "tw1", d_tw1[:, :])
            s_tb1 = wload([TE, 1], F32, "tb1", d_tb1[:, :])
            s_ones = wp.tile([1, TE], DT, tag="ones", name="ones")
            nc.gpsimd.memset(s_ones, 1.0)
            s_wz, s_wh, s_bz, s_bh = {}, {}, {}, {}
            for d in "fb":
                s_wz[d] = wload([IN_AUG, H], DT, f"wz{d}", d_wz[d][:, :])
                s_wh[d] = wload([IN_AUG, H], DT, f"wh{d}", d_wh[d][:, :])
                s_bz[d] = [wload([128, 1], F32, f"bz{d}{k}",
                                 d_bz[d][128 * k:128 * (k + 1), :]) for k in range(2)]
                s_bh[d] = [wload([128, 1], F32, f"bh{d}{k}",
                                 d_bh[d][128 * k:128 * (k + 1), :]) for k in range(2)]
            s_g1 = [wload([128, HH], DT, f"g1_{j}", d_g1[128 * j:128 * (j + 1), :])
                    for j in range(4)]
            s_g1te = wload([IN_AUG, HH], DT, "g1te", d_g1te[:, :])
            s_g2 = wload([HH, 1], DT, "g2", d_g2[:, :])
            s_gb1 = wload([HH, 1], F32, "gb1", d_gb1[:, :])

            copy = nc.any.tensor_copy if copy_engine == "any" else nc.vector.tensor_copy
            segs = L // psum_cols
            spc = psum_cols // FCH

            def gemm(rows, ktiles, consume):
                """Emit a [rows, L] GEMM in psum_cols segments; ktiles is a
                list of (lhsT, rhs_tile) accumulated along k; consume(ps, c0)
                evacuates each PSUM segment starting at column c0."""
                for seg in range(segs):
                    ps = pp.tile([rows, psum_cols], F32, tag="ps", name="ps")
                    for ki, (w, r) in enumerate(ktiles):
                        for ch in range(spc):
                            c0 = seg * psum_cols + ch * FCH
                            nc.tensor.matmul(
                                ps[:, ch * FCH:(ch + 1) * FCH], lhsT=w,
                                rhs=r[:, c0:c0 + FCH],
                                start=(ki == 0), stop=(ki == len(ktiles) - 1))
                    consume(ps, seg * psum_cols)

            for r in range(repeats):
                # stage 1: R = [relu(t*w1+b1)(64) ; x(2) ; ones(1)] per sequence
                rrs = []
                for bi in range(bs):
                    rr = mp.tile([IN_AUG, L], DT, tag="rr", name="rr", bufs=max(2, bs))
                    if r == 0:
                        nc.vector.memset(rr[TE:TE + 3, :], 1.0)
                    nc.sync.dma_start(out=rr[TE:TE + 2, :], in_=d_xT[bi])
                    if te1_mode == "act_pe":
                        t_row = mp.tile([1, L], DT, tag="t_row", name="t_row")
                        nc.sync.dma_start(out=t_row, in_=d_t[bi:bi + 1, :])

                        def te1_consume(ps, c0, rr=rr):
                            nc.scalar.activation(out=rr[0:TE, c0:c0 + psum_cols],
                                                 in_=ps,
                                                 func=mybir.ActivationFunctionType.Relu,
                                                 bias=s_tb1, scale=s_tw1)
                        gemm(TE, [(s_ones, t_row)], te1_consume)
                    else:
                        # DMA-broadcast t across 64 partitions; affine+relu on
                        # the (otherwise idle) engine named by te1_mode
                        eng = {"pool": nc.gpsimd, "dve": nc.vector}[te1_mode]
                        t_bc = mp.tile([TE, L], DT, tag="t_bc", name="t_bc")
                        trow_ap = d_t[bi:bi + 1, :]
                        for q in range(4):
                            nc.sync.dma_start(
                                out=t_bc[16 * q:16 * (q + 1), :],
                                in_=bass.AP(tensor=trow_ap.tensor,
                                            offset=trow_ap.offset,
                                            ap=[[0, 16], list(trow_ap.ap[-1])]))
                        eng.tensor_scalar(out=rr[0:TE, :], in0=t_bc,
                                          scalar1=s_tw1, scalar2=s_tb1,
                                          op0=mybir.AluOpType.mult,
                                          op1=mybir.AluOpType.add)
                        eng.tensor_relu(rr[0:TE, :], rr[0:TE, :])
                    rrs.append(rr)

                # stage 2: gates + scan per sequence; each head is
                # emitted after the NEXT sequence's gates so PE/ACT keep
                # dense work while the scans for the head finish
                def emit_head(bi, rr, h_out):
                        # gauss head: 5 accumulated k-tiles, relu, then k=128 -> 1
                        rt = mp.tile([HH, L], DT, tag="rt", name="rt")

                        def r_consume(ps, c0, rt=rt):
                            if r_engine == "act":
                                nc.scalar.activation(out=rt[:, c0:c0 + psum_cols], in_=ps,
                                                         func=mybir.ActivationFunctionType.Relu,
                                                         bias=s_gb1)
                            else:
                                nc.vector.tensor_scalar(out=rt[:, c0:c0 + psum_cols],
                                                            in0=ps, scalar1=s_gb1,
                                                            scalar2=0.0,
                                                            op0=mybir.AluOpType.add,
                                                            op1=mybir.AluOpType.max)
                        gemm(HH, [(s_g1[0], h_out["f"][0]), (s_g1[1], h_out["f"][1]),
                                  (s_g1[2], h_out["b"][0]), (s_g1[3], h_out["b"][1]),
                                  (s_g1te, rr)], r_consume)

                        orow = mp.tile([1, L], F32, tag="orow", name="orow")

                        def o_consume(ps, c0, orow=orow):
                            if orow_engine == "any":
                                copy(out=orow[:, c0:c0 + psum_cols], in_=ps)
                            else:
                                nc.vector.tensor_copy(out=orow[:, c0:c0 + psum_cols],
                                                          in_=ps)
                        gemm(1, [(s_g2, rt)], o_consume)
                        nc.sync.dma_start(out=d_out[bi:bi + 1, :], in_=orow)


                pending = None
                for bi in range(bs):
                    rr = rrs[bi]
                    h_out = {}
                    for d in "fb":
                        hs = []
                        for ph in range(2):
                            zt = mp.tile([128, L], DT, tag="zt", name="zt")

                            # a = 1-z = sigmoid(-(pre+bz)) straight from ACT
                            # (bz arrives pre-negated from the host)
                            def z_consume(ps, c0, zt=zt, d=d, ph=ph):
                                nc.scalar.activation(
                                    out=zt[:, c0:c0 + psum_cols], in_=ps,
                                    func=mybir.ActivationFunctionType.Sigmoid,
                                    scale=-1.0, bias=s_bz[d][ph])
                            gemm(128, [(s_wz[d][:, 128 * ph:128 * (ph + 1)], rr)],
                                 z_consume)
                            at = zt
                            ht = mp.tile([128, L], DT, tag="ht", name="ht")

                            def h_consume(ps, c0, ht=ht, d=d, ph=ph):
                                nc.scalar.activation(
                                    out=ht[:, c0:c0 + psum_cols], in_=ps,
                                    func=mybir.ActivationFunctionType.Tanh,
                                    bias=s_bh[d][ph])
                            gemm(128, [(s_wh[d][:, 128 * ph:128 * (ph + 1)], rr)],
                                 h_consume)
                            # -b = (a-1)*h~ in one DVE pass; the scan subtracts
                            bt = mp.tile([128, L], DT, tag="bt", name="bt")
                            nc.vector.scalar_tensor_tensor(
                                out=bt, in0=at, scalar=1.0, in1=ht,
                                op0=mybir.AluOpType.subtract,
                                op1=mybir.AluOpType.mult)

                            # linear scan; reference stores the pre-update
                            # state: outputs shift one step and the edge is 0.
                            hv = hp.tile([128, L], DT, tag=f"h{d}{ph}", name=f"h{d}{ph}")
                            if d == "f":
                                nc.gpsimd.memset(hv[:, 0:1], 0.0)
                                nc.vector.tensor_tensor_scan(
                                    out=hv[:, 1:L], data0=at[:, 0:L - 1],
                                    data1=bt[:, 0:L - 1], initial=0.0,
                                    op0=mybir.AluOpType.mult,
                                    op1=mybir.AluOpType.subtract)
                            else:
                                nc.gpsimd.memset(hv[:, L - 1:L], 0.0)
                                nc.vector.tensor_tensor_scan(
                                    out=_rev(hv, 0, L - 1), data0=_rev(at, 1, L - 1),
                                    data1=_rev(bt, 1, L - 1), initial=0.0,
                                    op0=mybir.AluOpType.mult,
                                    op1=mybir.AluOpType.subtract)
                            hs.append(hv)
                        h_out[d] = hs

                    if pending is not None:
                        emit_head(*pending)
                    pending = (bi, rr, h_out)
                emit_head(*pending)

    _spill_excess_waits(nc)
    return nc


def _host_prep(inputs):
    """Per-core input maps. The input projection and time-encoder second layer
    are composed into the gate/head weights (fp64) so the device operates
    directly on R = [te1_hidden(64); ones(1); x(2)]."""
    f = {k: np.asarray(v, np.float64) for k, v in inputs.items()}

    def dt(a):
        return np.ascontiguousarray(a.astype(np.float32).astype(NP_DT))

    def f32c(a):
        return np.ascontiguousarray(a.astype(np.float32))

    def gate_w(pw, pb, w):
        """(67,256) weight in the R basis for pre = (xc@[pw;pb]) @ w."""
        te_part = f["te_w2"] @ pw[2:66] @ w              # (64,256)
        x_part = pw[0:2] @ w                             # (2,256)
        ones_row = f["te_b2"] @ pw[2:66] @ w + pb @ w    # (256,)
        return np.concatenate([te_part, x_part, ones_row[None, :]], axis=0)

    common = {}
    for d, pw, pb in (("f", f["fproj_w"], f["fproj_b"]),
                      ("b", f["bproj_w"], f["bproj_b"])):
        common[f"wz{d}"] = dt(gate_w(pw, pb, f[f"{d}wz_w"]))
        common[f"wh{d}"] = dt(gate_w(pw, pb, f[f"{d}wh_w"]))
        common[f"bz{d}"] = f32c(-f[f"{d}wz_b"][:, None])
        common[f"bh{d}"] = f32c(f[f"{d}wh_b"][:, None])
    common["g1"] = dt(f["gh_w1"][0:2 * H])
    g1te = f["gh_w1"][2 * H:2 * H + TE]                  # (64,128)
    common["g1te"] = dt(np.concatenate(
        [f["te_w2"] @ g1te, np.zeros((2, HH)), (f["te_b2"] @ g1te)[None, :]], axis=0))
    common["g2"] = dt(f["gh_w2"])
    common["tw1"] = f32c(f["te_w1"].T)
    common["tb1"] = f32c(f["te_b1"][:, None])
    common["gb1"] = f32c(f["gh_b1"][:, None])
    in_maps = []
    for c in range(NCORES):
        sl = slice(BS * c, BS * (c + 1))
        m = dict(common)
        m["xT"] = dt(f["x"][sl].transpose(0, 2, 1))
        m["t"] = dt(f["t"][sl, :, 0])
        in_maps.append(m)
    return in_maps, float(f["gh_b2"][0])


_CACHE = {}


def _get_nc():
    if "nc" not in _CACHE:
        _CACHE["nc"] = _build_nc()
    return _CACHE["nc"]


def kernel(**inputs):
    nc = _get_nc()
    in_maps, gh_b2 = _host_prep(inputs)
    res = run_bass_kernel_spmd(nc, in_maps, list(range(NCORES)))
    out = np.empty((B, L, 1), np.float32)
    for c in range(NCORES):
        out[BS * c:BS * (c + 1), :, 0] = res.results[c]["out"] + gh_b2
    return out


def _build_sharded_exec(nc):
    """Non-donating clone of bass2jax.run_bass_via_pjrt's multi-core path so
    the executable can be launched repeatedly for timing."""
    import jax
    import concourse.mybir as mb
    from jax.experimental.shard_map import shard_map
    from jax.sharding import Mesh, PartitionSpec
    from concourse import bass2jax

    bass2jax.install_neuronx_cc_hook()
    part_name = nc.partition_id_tensor.name if nc.partition_id_tensor else None
    in_names, out_names, out_avals, zero_outs = [], [], [], []
    for alloc in nc.m.functions[0].allocations:
        if not isinstance(alloc, mb.MemoryLocationSet):
            continue
        name = alloc.memorylocations[0].name
        if alloc.kind == "ExternalInput":
            if name != part_name:
                in_names.append(name)
        elif alloc.kind == "ExternalOutput":
            shape = tuple(alloc.tensor_shape)
            dtype = mb.dt.np(alloc.dtype)
            out_names.append(name)
            out_avals.append(jax.core.ShapedArray(shape, dtype))
            zero_outs.append(np.zeros(shape, dtype))
    n_params = len(in_names)
    all_names = in_names + out_names
    if part_name is not None:
        all_names = all_names + [part_name]

    def _body(*args):
        operands = list(args)
        if part_name is not None:
            operands.append(bass2jax.partition_id_tensor())
        outs = bass2jax._bass_exec_p.bind(
            *operands,
            out_avals=tuple(out_avals),
            in_names=tuple(all_names),
            out_names=tuple(out_names),
            lowering_input_output_aliases=(),
            sim_require_finite=True,
            sim_require_nnan=True,
            nc=nc,
        )
        return tuple(outs)

    devices = jax.devices()[:NCORES]
    mesh = Mesh(np.asarray(devices), ("core",))
    nin = n_params + len(out_names)
    sharded = jax.jit(
        shard_map(_body, mesh=mesh,
                  in_specs=(PartitionSpec("core"),) * nin,
                  out_specs=(PartitionSpec("core"),) * len(out_names),
                  check_rep=False),
        keep_unused=True,
    )
    return sharded, mesh, in_names, out_names, zero_outs


def bench(inputs, r_lo=1, r_hi=33, blocks=300):
    """On-device per-iteration time (ns), free of launch overhead: build the
    kernel with the per-core work repeated r_lo x and r_hi x inside one NEFF
    and time ABBA-interleaved launches; the median paired difference divided
    by (r_hi - r_lo) cancels launch overhead and drift."""
    import jax
    from jax.sharding import NamedSharding, PartitionSpec

    in_maps, _ = _host_prep(inputs)

    def prep(nc):
        sharded, mesh, in_names, out_names, zero_outs = _build_sharded_exec(nc)
        sh = NamedSharding(mesh, PartitionSpec("core"))
        concat_in = [
            np.concatenate([np.asarray(in_maps[c][n]) for c in range(NCORES)], axis=0)
            for n in in_names
        ]
        concat_zero = [
            np.zeros((NCORES * z.shape[0], *z.shape[1:]), z.dtype) for z in zero_outs
        ]
        args = [jax.device_put(a, sh) for a in concat_in + concat_zero]
        return sharded, args

    s_lo, a_lo = prep(_build_nc(repeats=r_lo))
    s_hi, a_hi = prep(_build_nc(repeats=r_hi))

    def launch(s, a):
        t0 = time.perf_counter()
        jax.block_until_ready(s(*a))
        return time.perf_counter() - t0

    for _ in range(5):
        launch(s_lo, a_lo), launch(s_hi, a_hi)

    diffs = []
    for i in range(blocks):
        # ABBA: lo hi hi lo -> (B1+B2-A1-A2)/2 is drift-free
        A1 = launch(s_lo, a_lo)
        B1 = launch(s_hi, a_hi)
        B2 = launch(s_hi, a_hi)
        A2 = launch(s_lo, a_lo)
        diffs.append((B1 + B2 - A1 - A2) / 2)
    per = np.array(diffs) * 1e9 / (r_hi - r_lo)
    return float(np.median(per))


# revision 4
# speedup vs baseline: 1.1684x; 1.1684x over previous
"""Bi-directional minGRU Trainium2 kernel.

Full-input contract: kernel(**inputs) takes the unsharded numpy inputs from
reference.setup_inputs() and returns the full (B, L, 1) float32 output.

Sharding: data-parallel over batch B=32 across 8 NeuronCores (4 sequences per
core), parameters replicated. Per core, each sequence is processed in a
feature-on-partition / time-on-free layout:

  t_enc  : relu(t*w1+b1) via ScalarE (per-partition scale/bias), @w2 on PE
  xc     : [t_enc(64) ; x(2) ; ones(1)] -> 67 partitions (ones row folds the
           proj bias into the matmul)
  inp    : xc @ [proj_w;proj_b] on PE (2 p-halves)
  z,hb   : inp @ wz / wh on PE (2 k-tiles), sigmoid/tanh+bias on ScalarE
  a=1-z  : DVE tensor_scalar;  b=z*hb : DVE tensor_tensor
  scan   : DVE tensor_tensor_scan (state = a*state + b), shifted one step
           (reference stores pre-update state); backward direction runs the
           same scan through negative-stride APs
  head   : [h_fwd;h_bwd;t_enc] @ gh_w1 (5 k-tiles) + relu+bias on ScalarE,
           @ gh_w2 on PE; gh_b2 is added on host after the gather.
"""

import time

import numpy as np
import ml_dtypes

import concourse.bass as bass
import concourse.mybir as mybir
import concourse.tile as tile
from concourse.vector_clock import ScopedClock, VectorClock
from concourse.bass_utils import run_bass_kernel_spmd

# ---------------------------------------------------------------------------
# Workaround for a walrus codegen limit in this toolchain: the TileContext
# tail drain carries one sync-wait per live proc sem, but this walrus build
# rejects >2 sync waits on a Drain (CTRL_NO_STRUCT template). Re-emit the tail
# with the waits split across single-wait NOPs on the sync engine (same-engine
# program order preserves the semantics), followed by a wait-free drain.
# ---------------------------------------------------------------------------


def _patched_drain_and_barrier(self, tick_clock, wait_clock):
    nc = self.nc
    vals = list(tick_clock.global_clock)
    n = len(vals)
    for i, v in enumerate(vals):
        if v > 0:
            partial = [0] * n
            partial[i] = v
            nop = nc.sync.nop()
            wait_clock.add_sem_waits(nop.ins, ScopedClock({None: VectorClock(partial)}))
    nc.sync.drain()
    nc.all_engine_barrier()
    assert self.sems is not None
    popped = nc._tile_sem_poison_stack.pop()
    assert popped is self._sem_poison
    nc.clear_and_free_semaphores(list(self.sems.allocated().values()))
    nc.all_engine_barrier()


tile.TileContext._drain_and_barrier = _patched_drain_and_barrier


def _spill_excess_waits(nc, maxw=1):
    """Split instructions carrying more than `maxw` sem waits: the excess
    waits move onto NoOps inserted just before, on the same engine (same-
    engine program order keeps the semantics identical)."""
    for bb in nc.m.functions[0].blocks:
        new = []
        for inst in bb.instructions:
            si = inst.sync_info
            if si is not None and si.on_wait is not None and len(si.on_wait) > maxw:
                waits = list(si.on_wait)
                excess, keep = waits[:-maxw], waits[-maxw:]
                for j, w in enumerate(excess):
                    nop = mybir.InstNoOp(
                        name=f"{inst.name}_ws{j}",
                        engine=inst.engine,
                        ins=[],
                        outs=[],
                        sync_info=mybir.SyncInfo(on_wait=[w], on_update=[]),
                    )
                    nc.register_instruction(nop)
                    new.append(nop)
                si.on_wait = keep
            new.append(inst)
        if len(new) != len(bb.instructions):
            _replace_block_instructions(bb, new)


def _replace_block_instructions(bb, new):
    try:
        bb.instructions = new
    except Exception:
        while len(bb.instructions):
            bb.instructions.pop()
        for inst in new:
            bb.add_instruction(inst)

# ---------------------------------------------------------------------------

B, L, H, TE = 32, 2048, 256, 64
NCORES = 8
BS = B // NCORES           # sequences per core
HH = 128                   # gauss head hidden
IN_AUG = TE + 2 + 1        # xc rows: t_enc(64) + x(2) + ones(1)
F32 = mybir.dt.float32

DT = mybir.dt.bfloat16     # matmul/activation storage dtype
NP_DT = ml_dtypes.bfloat16

FCH = 512                  # matmul moving-operand chunk (one PSUM bank fp32)
NCH = L // FCH


def _rev(t, cols, ncols):
    """Reversed-free-dim view of tile AP t over columns [cols, cols+ncols)."""
    return bass.AP(
        tensor=t.tensor,
        offset=t.offset + cols + ncols - 1,
        ap=[list(t.ap[0]), [-1, ncols]],
    )


def _build_nc(bs=BS, repeats=1, psum_cols=1024, psum_bufs=4, mp_bufs=2, hp_bufs=3,
              copy_engine="any", ab_engine="vector", bwd_scan_engine="vector",
              te1_mode="dve", r_engine="act", orow_engine="any"):
    nc = bass.Bass("TRN2", target_bir_lowering=False, debug=False, num_devices=NCORES)

    d_xT = nc.dram_tensor("xT", [bs, 2, L], DT, kind="ExternalInput")
    d_t = nc.dram_tensor("t", [bs, L], DT, kind="ExternalInput")
    # gate weights with the input projection and time-encoder composed in
    # (host-side): operate directly on R = [te1_hidden(64); x(2); ones(1)]
    d_wz = {d: nc.dram_tensor(f"wz{d}", [IN_AUG, H], DT, kind="ExternalInput")
            for d in "fb"}
    d_wh = {d: nc.dram_tensor(f"wh{d}", [IN_AUG, H], DT, kind="ExternalInput")
            for d in "fb"}
    d_bz = {d: nc.dram_tensor(f"bz{d}", [H, 1], F32, kind="ExternalInput")
            for d in "fb"}
    d_bh = {d: nc.dram_tensor(f"bh{d}", [H, 1], F32, kind="ExternalInput")
            for d in "fb"}
    d_g1 = nc.dram_tensor("g1", [2 * H, HH], DT, kind="ExternalInput")
    d_g1te = nc.dram_tensor("g1te", [IN_AUG, HH], DT, kind="ExternalInput")
    d_g2 = nc.dram_tensor("g2", [HH, 1], DT, kind="ExternalInput")
    d_tw1 = nc.dram_tensor("tw1", [TE, 1], F32, kind="ExternalInput")
    d_tb1 = nc.dram_tensor("tb1", [TE, 1], F32, kind="ExternalInput")
    d_gb1 = nc.dram_tensor("gb1", [HH, 1], F32, kind="ExternalInput")
    d_out = nc.dram_tensor("out", [bs, L], F32, kind="ExternalOutput")

    with tile.TileContext(nc) as tc:
        with (
            tc.tile_pool(name="wpool", bufs=1) as wp,
            tc.tile_pool(name="mpool", bufs=mp_bufs) as mp,
            tc.tile_pool(name="hpool", bufs=hp_bufs) as hp,
            tc.tile_pool(name="psum", bufs=psum_bufs, space="PSUM") as pp,
        ):
            # ---- replicated weights, loaded once ----
            # DMAs round-robin across engine queues, ordered by first use so
            # the first batch's gates aren't gated on the full weight load.
            _eng = [nc.sync, nc.gpsimd, nc.scalar]
            _ei = [0]

            def wload(shape, dtype, tag, src_ap):
                t = wp.tile(shape, dtype, tag=tag, name=tag)
                _eng[_ei[0] % len(_eng)].dma_start(out=t, in_=src_ap)
                _ei[0] += 1
                return t

            s_tw1 = wload([TE, 1], F32, "tw1", d_tw1[:, :])
            s_tb1 = wload([TE, 1], F32, "tb1", d_tb1[:, :])
            s_ones = wp.tile([1, TE], DT, tag="ones", name="ones")
            nc.gpsimd.memset(s_ones, 1.0)
            s_wz, s_wh, s_bz, s_bh = {}, {}, {}, {}
            for d in "fb":
                s_wz[d] = wload([IN_AUG, H], DT, f"wz{d}", d_wz[d][:, :])
                s_wh[d] = wload([IN_AUG, H], DT, f"wh{d}", d_wh[d][:, :])
                s_bz[d] = [wload([128, 1], F32, f"bz{d}{k}",
                                 d_bz[d][128 * k:128 * (k + 1), :]) for k in range(2)]
                s_bh[d] = [wload([128, 1], F32, f"bh{d}{k}",
                                 d_bh[d][128 * k:128 * (k + 1), :]) for k in range(2)]
            s_g1 = [wload([128, HH], DT, f"g1_{j}", d_g1[128 * j:128 * (j + 1), :])
                    for j in range(4)]
            s_g1te = wload([IN_AUG, HH], DT, "g1te", d_g1te[:, :])
            s_g2 = wload([HH, 1], DT, "g2", d_g2[:, :])
            s_gb1 = wload([HH, 1], F32, "gb1", d_gb1[:, :])

            copy = nc.any.tensor_copy if copy_engine == "any" else nc.vector.tensor_copy
            segs = L // psum_cols
            spc = psum_cols // FCH

            def gemm(rows, ktiles, consume):
                """Emit a [rows, L] GEMM in psum_cols segments; ktiles is a
                list of (lhsT, rhs_tile) accumulated along k; consume(ps, c0)
                evacuates each PSUM segment starting at column c0."""
                for seg in range(segs):
                    ps = pp.tile([rows, psum_cols], F32, tag="ps", name="ps")
                    for ki, (w, r) in enumerate(ktiles):
                        for ch in range(spc):
                            c0 = seg * psum_cols + ch * FCH
                            nc.tensor.matmul(
                                ps[:, ch * FCH:(ch + 1) * FCH], lhsT=w,
                                rhs=r[:, c0:c0 + FCH],
                                start=(ki == 0), stop=(ki == len(ktiles) - 1))
                    consume(ps, seg * psum_cols)

            for r in range(repeats):
                # stage 1: R = [relu(t*w1+b1)(64) ; x(2) ; ones(1)] per sequence
                rrs = []
                for bi in range(bs):
                    rr = mp.tile([IN_AUG, L], DT, tag="rr", name="rr", bufs=max(2, bs))
                    if r == 0:
                        nc.vector.memset(rr[TE:TE + 3, :], 1.0)
                    nc.sync.dma_start(out=rr[TE:TE + 2, :], in_=d_xT[bi])
                    if te1_mode == "act_pe":
                        t_row = mp.tile([1, L], DT, tag="t_row", name="t_row")
                        nc.sync.dma_start(out=t_row, in_=d_t[bi:bi + 1, :])

                        def te1_consume(ps, c0, rr=rr):
                            nc.scalar.activation(out=rr[0:TE, c0:c0 + psum_cols],
                                                 in_=ps,
                                                 func=mybir.ActivationFunctionType.Relu,
                                                 bias=s_tb1, scale=s_tw1)
                        gemm(TE, [(s_ones, t_row)], te1_consume)
                    else:
                        # DMA-broadcast t across 64 partitions; affine+relu on
                        # the (otherwise idle) engine named by te1_mode
                        eng = {"pool": nc.gpsimd, "dve": nc.vector}[te1_mode]
                        t_bc = mp.tile([TE, L], DT, tag="t_bc", name="t_bc")
                        trow_ap = d_t[bi:bi + 1, :]
                        nc.sync.dma_start(
                            out=t_bc[0:TE, :],
                            in_=bass.AP(tensor=trow_ap.tensor,
                                        offset=trow_ap.offset,
                                        ap=[[0, TE], list(trow_ap.ap[-1])]))
                        eng.tensor_scalar(out=rr[0:TE, :], in0=t_bc,
                                          scalar1=s_tw1, scalar2=s_tb1,
                                          op0=mybir.AluOpType.mult,
                                          op1=mybir.AluOpType.add)
                        eng.tensor_relu(rr[0:TE, :], rr[0:TE, :])
                    rrs.append(rr)

                # stage 2: gates + scan per sequence; each head is
                # emitted after the NEXT sequence's gates so PE/ACT keep
                # dense work while the scans for the head finish
                def emit_head(bi, rr, h_out):
                        # gauss head: 5 accumulated k-tiles, relu, then k=128 -> 1
                        rt = mp.tile([HH, L], DT, tag="rt", name="rt")

                        def r_consume(ps, c0, rt=rt):
                            if r_engine == "act":
                                nc.scalar.activation(out=rt[:, c0:c0 + psum_cols], in_=ps,
                                                         func=mybir.ActivationFunctionType.Relu,
                                                         bias=s_gb1)
                            else:
                                nc.vector.tensor_scalar(out=rt[:, c0:c0 + psum_cols],
                                                            in0=ps, scalar1=s_gb1,
                                                            scalar2=0.0,
                                                            op0=mybir.AluOpType.add,
                                                            op1=mybir.AluOpType.max)
                        gemm(HH, [(s_g1[0], h_out["f"][0]), (s_g1[1], h_out["f"][1]),
                                  (s_g1[2], h_out["b"][0]), (s_g1[3], h_out["b"][1]),
                                  (s_g1te, rr)], r_consume)

                        orow = mp.tile([1, L], F32, tag="orow", name="orow")

                        def o_consume(ps, c0, orow=orow):
                            if orow_engine == "any":
                                copy(out=orow[:, c0:c0 + psum_cols], in_=ps)
                            else:
                                nc.vector.tensor_copy(out=orow[:, c0:c0 + psum_cols],
                                                          in_=ps)
                        gemm(1, [(s_g2, rt)], o_consume)
                        nc.sync.dma_start(out=d_out[bi:bi + 1, :], in_=orow)


                pending = None
                for bi in range(bs):
                    rr = rrs[bi]
                    h_out = {}
                    for d in "fb":
                        hs = []
                        for ph in range(2):
                            zt = mp.tile([128, L], DT, tag="zt", name="zt")

                            # a = 1-z = sigmoid(-(pre+bz)) straight from ACT
                            # (bz arrives pre-negated from the host)
                            def z_consume(ps, c0, zt=zt, d=d, ph=ph):
                                nc.scalar.activation(
                                    out=zt[:, c0:c0 + psum_cols], in_=ps,
                                    func=mybir.ActivationFunctionType.Sigmoid,
                                    scale=-1.0, bias=s_bz[d][ph])
                            gemm(128, [(s_wz[d][:, 128 * ph:128 * (ph + 1)], rr)],
                                 z_consume)
                            at = zt
                            ht = mp.tile([128, L], DT, tag="ht", name="ht")

                            def h_consume(ps, c0, ht=ht, d=d, ph=ph):
                                nc.scalar.activation(
                                    out=ht[:, c0:c0 + psum_cols], in_=ps,
                                    func=mybir.ActivationFunctionType.Tanh,
                                    bias=s_bh[d][ph])
                            gemm(128, [(s_wh[d][:, 128 * ph:128 * (ph + 1)], rr)],
                                 h_consume)
                            # -b = (a-1)*h~ in one DVE pass; the scan subtracts
                            bt = mp.tile([128, L], DT, tag="bt", name="bt")
                            nc.vector.scalar_tensor_tensor(
                                out=bt, in0=at, scalar=1.0, in1=ht,
                                op0=mybir.AluOpType.subtract,
                                op1=mybir.AluOpType.mult)

                            # linear scan; reference stores the pre-update
                            # state: outputs shift one step and the edge is 0.
                            hv = hp.tile([128, L], DT, tag=f"h{d}{ph}", name=f"h{d}{ph}")
                            if d == "f":
                                nc.gpsimd.memset(hv[:, 0:1], 0.0)
                                nc.vector.tensor_tensor_scan(
                                    out=hv[:, 1:L], data0=at[:, 0:L - 1],
                                    data1=bt[:, 0:L - 1], initial=0.0,
                                    op0=mybir.AluOpType.mult,
                                    op1=mybir.AluOpType.subtract)
                            else:
                                nc.gpsimd.memset(hv[:, L - 1:L], 0.0)
                                nc.vector.tensor_tensor_scan(
                                    out=_rev(hv, 0, L - 1), data0=_rev(at, 1, L - 1),
                                    data1=_rev(bt, 1, L - 1), initial=0.0,
                                    op0=mybir.AluOpType.mult,
                                    op1=mybir.AluOpType.subtract)
                            hs.append(hv)
                        h_out[d] = hs

                    if pending is not None:
                        emit_head(*pending)
                    pending = (bi, rr, h_out)
                emit_head(*pending)

    _spill_excess_waits(nc)
    return nc


def _host_prep(inputs):
    """Per-core input maps. The input projection and time-encoder second layer
    are composed into the gate/head weights (fp64) so the device operates
    directly on R = [te1_hidden(64); ones(1); x(2)]."""
    f = {k: np.asarray(v, np.float64) for k, v in inputs.items()}

    def dt(a):
        return np.ascontiguousarray(a.astype(np.float32).astype(NP_DT))

    def f32c(a):
        return np.ascontiguousarray(a.astype(np.float32))

    def gate_w(pw, pb, w):
        """(67,256) weight in the R basis for pre = (xc@[pw;pb]) @ w."""
        te_part = f["te_w2"] @ pw[2:66] @ w              # (64,256)
        x_part = pw[0:2] @ w                             # (2,256)
        ones_row = f["te_b2"] @ pw[2:66] @ w + pb @ w    # (256,)
        return np.concatenate([te_part, x_part, ones_row[None, :]], axis=0)

    common = {}
    for d, pw, pb in (("f", f["fproj_w"], f["fproj_b"]),
                      ("b", f["bproj_w"], f["bproj_b"])):
        common[f"wz{d}"] = dt(gate_w(pw, pb, f[f"{d}wz_w"]))
        common[f"wh{d}"] = dt(gate_w(pw, pb, f[f"{d}wh_w"]))
        common[f"bz{d}"] = f32c(-f[f"{d}wz_b"][:, None])
        common[f"bh{d}"] = f32c(f[f"{d}wh_b"][:, None])
    common["g1"] = dt(f["gh_w1"][0:2 * H])
    g1te = f["gh_w1"][2 * H:2 * H + TE]                  # (64,128)
    common["g1te"] = dt(np.concatenate(
        [f["te_w2"] @ g1te, np.zeros((2, HH)), (f["te_b2"] @ g1te)[None, :]], axis=0))
    common["g2"] = dt(f["gh_w2"])
    common["tw1"] = f32c(f["te_w1"].T)
    common["tb1"] = f32c(f["te_b1"][:, None])
    common["gb1"] = f32c(f["gh_b1"][:, None])
    in_maps = []
    for c in range(NCORES):
        sl = slice(BS * c, BS * (c + 1))
        m = dict(common)
        m["xT"] = dt(f["x"][sl].transpose(0, 2, 1))
        m["t"] = dt(f["t"][sl, :, 0])
        in_maps.append(m)
    return in_maps, float(f["gh_b2"][0])


_CACHE = {}


def _get_nc():
    if "nc" not in _CACHE:
        _CACHE["nc"] = _build_nc()
    return _CACHE["nc"]


def kernel(**inputs):
    nc = _get_nc()
    in_maps, gh_b2 = _host_prep(inputs)
    res = run_bass_kernel_spmd(nc, in_maps, list(range(NCORES)))
    out = np.empty((B, L, 1), np.float32)
    for c in range(NCORES):
        out[BS * c:BS * (c + 1), :, 0] = res.results[c]["out"] + gh_b2
    return out


def _build_sharded_exec(nc):
    """Non-donating clone of bass2jax.run_bass_via_pjrt's multi-core path so
    the executable can be launched repeatedly for timing."""
    import jax
    import concourse.mybir as mb
    from jax.experimental.shard_map import shard_map
    from jax.sharding import Mesh, PartitionSpec
    from concourse import bass2jax

    bass2jax.install_neuronx_cc_hook()
    part_name = nc.partition_id_tensor.name if nc.partition_id_tensor else None
    in_names, out_names, out_avals, zero_outs = [], [], [], []
    for alloc in nc.m.functions[0].allocations:
        if not isinstance(alloc, mb.MemoryLocationSet):
            continue
        name = alloc.memorylocations[0].name
        if alloc.kind == "ExternalInput":
            if name != part_name:
                in_names.append(name)
        elif alloc.kind == "ExternalOutput":
            shape = tuple(alloc.tensor_shape)
            dtype = mb.dt.np(alloc.dtype)
            out_names.append(name)
            out_avals.append(jax.core.ShapedArray(shape, dtype))
            zero_outs.append(np.zeros(shape, dtype))
    n_params = len(in_names)
    all_names = in_names + out_names
    if part_name is not None:
        all_names = all_names + [part_name]

    def _body(*args):
        operands = list(args)
        if part_name is not None:
            operands.append(bass2jax.partition_id_tensor())
        outs = bass2jax._bass_exec_p.bind(
            *operands,
            out_avals=tuple(out_avals),
            in_names=tuple(all_names),
            out_names=tuple(out_names),
            lowering_input_output_aliases=(),
            sim_require_finite=True,
            sim_require_nnan=True,
            nc=nc,
        )
        return tuple(outs)

    devices = jax.devices()[:NCORES]
    mesh = Mesh(np.asarray(devices), ("core",))
    nin = n_params + len(out_names)
    sharded = jax.jit(
        shard_map(_body, mesh=mesh,
                  in_specs=(PartitionSpec("core"),) * nin,
                  out_specs=(PartitionSpec("core"),) * len(out_names),
                  check_rep=False),
        keep_unused=True,
    )
    return sharded, mesh, in_names, out_names, zero_outs


def bench(inputs, r_lo=1, r_hi=33, blocks=300):
    """On-device per-iteration time (ns), free of launch overhead: build the
    kernel with the per-core work repeated r_lo x and r_hi x inside one NEFF
    and time ABBA-interleaved launches; the median paired difference divided
    by (r_hi - r_lo) cancels launch overhead and drift."""
    import jax
    from jax.sharding import NamedSharding, PartitionSpec

    in_maps, _ = _host_prep(inputs)

    def prep(nc):
        sharded, mesh, in_names, out_names, zero_outs = _build_sharded_exec(nc)
        sh = NamedSharding(mesh, PartitionSpec("core"))
        concat_in = [
            np.concatenate([np.asarray(in_maps[c][n]) for c in range(NCORES)], axis=0)
            for n in in_names
        ]
        concat_zero = [
            np.zeros((NCORES * z.shape[0], *z.shape[1:]), z.dtype) for z in zero_outs
        ]
        args = [jax.device_put(a, sh) for a in concat_in + concat_zero]
        return sharded, args

    s_lo, a_lo = prep(_build_nc(repeats=r_lo))
    s_hi, a_hi = prep(_build_nc(repeats=r_hi))

    def launch(s, a):
        t0 = time.perf_counter()
        jax.block_until_ready(s(*a))
        return time.perf_counter() - t0

    for _ in range(5):
        launch(s_lo, a_lo), launch(s_hi, a_hi)

    diffs = []
    for i in range(blocks):
        # ABBA: lo hi hi lo -> (B1+B2-A1-A2)/2 is drift-free
        A1 = launch(s_lo, a_lo)
        B1 = launch(s_hi, a_hi)
        B2 = launch(s_hi, a_hi)
        A2 = launch(s_lo, a_lo)
        diffs.append((B1 + B2 - A1 - A2) / 2)
    per = np.array(diffs) * 1e9 / (r_hi - r_lo)
    return float(np.median(per))
